# revision 4
# baseline (speedup 1.0000x reference)
"""Locally-connected 3x3 block (LCBlock) Trainium2 kernel.

Computes out = ELU(einsum('ocdkij,bcdkij->boij', weights, unfold(x)))
for x:[16,32,64,64] f32, weights:[32,32,3,3,64,64] f32.

Strategy (8 NeuronCores, SPMD, no collectives):
  - Spatially shard H=64 into 8 strips of 8 rows; each core gets its strip's
    per-position weights (they shard perfectly) and a 10-row halo'd slab of x.
  - Per position p=(y,x) the LC contraction is a tiny matmul
    [B=16, CK=288] x [CK=288, O=32].  We run it on the PE as 3 PSUM-accumulated
    matmuls (one per dj kernel column): lhsT = patch [K=96=(3di x 32c), M=16b]
    (cheap LDWEIGHTS: cost scales with columns=16), rhs = weights
    [96, 32o] (the big tensor streams as the moving operand).  4 positions run
    concurrently in the 4 PE column-groups via tile_position.
  - bf16 operands (fp32 PSUM accumulation) halve the HBM roofline.
  - ELU = max(x, exp(min(x,0))-1): 2 DVE ops + 1 ACT op per row-wave.
Host side packs/scatters inputs and gathers the 8 output strips.
"""

import os
import sys

import numpy as np

for _p in ("/opt/trn_rl_repo", "/root/.axon_site/_ro/trn_rl_repo"):
    if os.path.isdir(_p) and _p not in sys.path:
        sys.path.insert(0, _p)

import ml_dtypes

import concourse.bacc as bacc
import concourse.mybir as mybir
import concourse.tile as tile
from concourse.bass_interp import get_hw_module
from concourse.bass_utils import run_bass_kernel_spmd

BF16 = ml_dtypes.bfloat16

# Problem shape (hardcoded per contract).
B, C, O, H, W = 16, 32, 32, 64, 64
NCORES = 8
HL = H // NCORES  # local rows per core
KW = 3  # conv kernel size
PART = KW * C  # 96 partitions: (di, c)
XW = W + 2  # padded row width
XFREE = HL * XW * B  # x slab free elems/partition
WCH = 4 * 16 * KW * O  # weight elems/partition per row-wave (j, pbl, dj, o)
WFREE = HL * WCH
OUTF = HL * 16 * O  # out free elems/partition: (w, pbl, o)

_CACHE = {}


def _build(hw=True):
    nc = bacc.Bacc(
        "TRN2", target_bir_lowering=False, debug=False, num_devices=NCORES
    )
    xs_d = nc.dram_tensor("xs", [PART, XFREE], mybir.dt.bfloat16, kind="ExternalInput")
    w_d = nc.dram_tensor("w", [PART, WFREE], mybir.dt.bfloat16, kind="ExternalInput")
    out_d = nc.dram_tensor("out", [4, 16, OUTF], mybir.dt.float32, kind="ExternalOutput")

    with tile.TileContext(nc) as tc:
        with (
            tc.tile_pool(name="xp", bufs=1) as xp,
            tc.tile_pool(name="wp", bufs=3) as wp,
            tc.tile_pool(name="pp", bufs=3, space="PSUM") as pp,
            tc.tile_pool(name="op", bufs=1) as op,
            tc.tile_pool(name="tp", bufs=2) as tp,
        ):
            x_t = xp.tile([PART, XFREE], mybir.dt.bfloat16)
            nc.sync.dma_start(x_t[:], xs_d[:])
            out_t = op.tile([128, OUTF], mybir.dt.float32)

            for wv in range(HL):  # one image row per wave
                w_t = wp.tile([PART, WCH], mybir.dt.bfloat16, tag="w")
                nc.sync.dma_start(w_t[:], w_d[:][:, wv * WCH:(wv + 1) * WCH])
                ps = pp.tile([128, 512], mybir.dt.float32, tag="ps")
                # init rows the col-tiled matmuls never touch (ELU reads all 128)
                nc.vector.memset(ps[:], 0.0)
                for pbl in range(16):
                    for j in range(4):
                        roff = (j * 16 + pbl) * 3 * O
                        for dj in range(KW):
                            lo = (wv * XW + pbl * 4 + j + dj) * B
                            nc.tensor.matmul(
                                ps[32 * j:32 * j + B, pbl * 32:(pbl + 1) * 32],
                                x_t[:, lo:lo + B],
                                w_t[:, roff + dj * O:roff + (dj + 1) * O],
                                start=(dj == 0),
                                stop=(dj == KW - 1),
                                tile_position=(0, 32 * j),
                            )
                # ELU: out = max(psum, exp(min(psum, 0)) - 1)
                t1 = tp.tile([128, 512], mybir.dt.float32, tag="t1")
                nc.vector.tensor_scalar_min(t1[:], ps[:], 0.0)
                nc.scalar.activation(t1[:], t1[:], mybir.ActivationFunctionType.Exp)
                nc.vector.scalar_tensor_tensor(
                    out_t[:, wv * 512:(wv + 1) * 512],
                    t1[:],
                    -1.0,
                    ps[:],
                    op0=mybir.AluOpType.add,
                    op1=mybir.AluOpType.max,
                )
            oap = out_d.ap()
            for j in range(4):
                nc.sync.dma_start(oap[j], out_t[32 * j:32 * j + 16, :])

    nc.compile()
    if hw:
        nc.m = get_hw_module(nc.m)
    return nc


def _pack_inputs(x, weights):
    """Host-side scatter: per-core bf16 slabs."""
    xpad = np.pad(x, ((0, 0), (0, 0), (1, 1), (1, 1))).astype(BF16)  # [B,C,66,66]
    wb = np.asarray(weights).astype(BF16)  # [O,C,3,3,H,W]
    in_maps = []
    for k in range(NCORES):
        # x slab: [di*32+c, y, xx, b] = xpad[b, c, 8k+y+di, xx]
        slabs = [
            np.transpose(xpad[:, :, 8 * k + di:8 * k + di + HL, :], (1, 2, 3, 0))
            for di in range(KW)
        ]
        xs_k = np.ascontiguousarray(np.stack(slabs, 0)).reshape(PART, XFREE)
        # weights: [di*32+c, w, j, pbl, dj, o] = W[o, c, di, dj, 8k+w, pbl*4+j]
        wc = wb[:, :, :, :, 8 * k:8 * (k + 1), :].reshape(O, C, KW, KW, HL, 16, 4)
        w_k = np.ascontiguousarray(
            np.transpose(wc, (2, 1, 4, 6, 5, 3, 0))
        ).reshape(PART, WFREE)
        in_maps.append({"xs": xs_k, "w": w_k})
    return in_maps


def _unpack_outputs(results):
    out = np.empty((B, O, H, W), dtype=np.float32)
    for k in range(NCORES):
        arr = results[k]["out"].reshape(4, 16, HL, 16, O)  # [j, b, w, pbl, o]
        strip = np.transpose(arr, (1, 4, 2, 3, 0)).reshape(B, O, HL, W)
        out[:, :, 8 * k:8 * (k + 1), :] = strip
    return out


def run(x, weights, trace=False):
    if "nc" not in _CACHE:
        _CACHE["nc"] = _build()
    nc = _CACHE["nc"]
    in_maps = _pack_inputs(np.asarray(x), np.asarray(weights))
    res = run_bass_kernel_spmd(nc, in_maps, list(range(NCORES)), trace=trace)
    return _unpack_outputs(res.results), res


def kernel(x, weights):
    out, _ = run(x, weights)
    return out


# revision 6
# speedup vs baseline: 12.2179x; 12.2179x over previous
"""Locally-connected 3x3 block (LCBlock) Trainium2 kernel.

Computes out = ELU(einsum('ocdkij,bcdkij->boij', weights, unfold(x)))
for x:[16,32,64,64] f32, weights:[32,32,3,3,64,64] f32.

Strategy (8 NeuronCores, SPMD, no collectives):
  - Spatially shard H=64 into 8 strips of 8 rows; each core gets its strip's
    per-position weights (they shard perfectly) and a 10-row halo'd slab of x.
  - Per position p=(y,x) the LC contraction is a tiny matmul
    [B=16, CK=288] x [CK=288, O=32].  We run it on the PE as 3 PSUM-accumulated
    matmuls (one per dj kernel column): lhsT = patch [K=96=(3di x 32c), M=16b]
    (cheap LDWEIGHTS: cost scales with columns=16), rhs = weights
    [96, 32o] (the big tensor streams as the moving operand).  4 positions run
    concurrently in the 4 PE column-groups via tile_position.
  - bf16 operands (fp32 PSUM accumulation) halve the HBM roofline.
  - ELU = max(x, exp(min(x,0))-1): 2 DVE ops + 1 ACT op per row-wave.
Host side packs/scatters inputs and gathers the 8 output strips.
"""

import os
import sys

import numpy as np

for _p in ("/opt/trn_rl_repo", "/root/.axon_site/_ro/trn_rl_repo"):
    if os.path.isdir(_p) and _p not in sys.path:
        sys.path.insert(0, _p)

import ml_dtypes

import concourse.bacc as bacc
import concourse.mybir as mybir
import concourse.tile as tile
from concourse.bass_interp import get_hw_module
from concourse.bass_utils import run_bass_kernel_spmd

BF16 = ml_dtypes.bfloat16

# Problem shape (hardcoded per contract).
B, C, O, H, W = 16, 32, 32, 64, 64
NCORES = 8
HL = H // NCORES  # local rows per core
KW = 3  # conv kernel size
PART = KW * C  # 96 partitions: (di, c)
XW = W + 2  # padded row width
XFREE = HL * XW * B  # x slab free elems/partition
WCH = 4 * 16 * KW * O  # weight elems/partition per row-wave (j, pbl, dj, o)
WFREE = HL * WCH
OUTF = HL * 16 * O  # out free elems/partition: (w, pbl, o)

_CACHE = {}


def _build(hw=True, reps=1):
    nc = bacc.Bacc(
        "TRN2", target_bir_lowering=False, debug=False, num_devices=NCORES
    )
    xs_d = nc.dram_tensor("xs", [PART, XFREE], mybir.dt.bfloat16, kind="ExternalInput")
    w_d = nc.dram_tensor("w", [PART, WFREE], mybir.dt.bfloat16, kind="ExternalInput")
    out_d = nc.dram_tensor("out", [4, 16, OUTF], mybir.dt.float32, kind="ExternalOutput")

    with tile.TileContext(nc) as tc:
        with (
            tc.tile_pool(name="xp", bufs=1) as xp,
            tc.tile_pool(name="wp", bufs=3) as wp,
            tc.tile_pool(name="pp", bufs=3, space="PSUM") as pp,
            tc.tile_pool(name="op", bufs=1) as op,
            tc.tile_pool(name="tp", bufs=2) as tp,
        ):
          for _rep in range(reps):
            x_t = xp.tile([PART, XFREE], mybir.dt.bfloat16, tag="x")
            nc.sync.dma_start(x_t[:], xs_d[:])
            out_t = op.tile([128, OUTF], mybir.dt.float32, tag="o")

            for wv in range(HL):  # one image row per wave
                w_t = wp.tile([PART, WCH], mybir.dt.bfloat16, tag="w")
                nc.sync.dma_start(w_t[:], w_d[:][:, wv * WCH:(wv + 1) * WCH])
                ps = pp.tile([128, 512], mybir.dt.float32, tag="ps")
                # init rows the col-tiled matmuls never touch (ELU reads all 128)
                nc.vector.memset(ps[:], 0.0)
                for pbl in range(16):
                    for j in range(4):
                        roff = (j * 16 + pbl) * 3 * O
                        for dj in range(KW):
                            lo = (wv * XW + pbl * 4 + j + dj) * B
                            nc.tensor.matmul(
                                ps[32 * j:32 * j + B, pbl * 32:(pbl + 1) * 32],
                                x_t[:, lo:lo + B],
                                w_t[:, roff + dj * O:roff + (dj + 1) * O],
                                start=(dj == 0),
                                stop=(dj == KW - 1),
                                tile_position=(0, 32 * j),
                            )
                # ELU: out = max(psum, exp(min(psum, 0)) - 1)
                t1 = tp.tile([128, 512], mybir.dt.float32, tag="t1")
                nc.vector.tensor_scalar_min(t1[:], ps[:], 0.0)
                nc.scalar.activation(t1[:], t1[:], mybir.ActivationFunctionType.Exp)
                nc.vector.scalar_tensor_tensor(
                    out_t[:, wv * 512:(wv + 1) * 512],
                    t1[:],
                    -1.0,
                    ps[:],
                    op0=mybir.AluOpType.add,
                    op1=mybir.AluOpType.max,
                )
            oap = out_d.ap()
            for j in range(4):
                nc.sync.dma_start(oap[j], out_t[32 * j:32 * j + 16, :])

    nc.compile()
    if hw:
        nc.m = get_hw_module(nc.m)
    return nc


def _pack_inputs(x, weights):
    """Host-side scatter: per-core bf16 slabs."""
    xpad = np.pad(x, ((0, 0), (0, 0), (1, 1), (1, 1))).astype(BF16)  # [B,C,66,66]
    wb = np.asarray(weights).astype(BF16)  # [O,C,3,3,H,W]
    in_maps = []
    for k in range(NCORES):
        # x slab: [di*32+c, y, xx, b] = xpad[b, c, 8k+y+di, xx]
        slabs = [
            np.transpose(xpad[:, :, 8 * k + di:8 * k + di + HL, :], (1, 2, 3, 0))
            for di in range(KW)
        ]
        xs_k = np.ascontiguousarray(np.stack(slabs, 0)).reshape(PART, XFREE)
        # weights: [di*32+c, w, j, pbl, dj, o] = W[o, c, di, dj, 8k+w, pbl*4+j]
        wc = wb[:, :, :, :, 8 * k:8 * (k + 1), :].reshape(O, C, KW, KW, HL, 16, 4)
        w_k = np.ascontiguousarray(
            np.transpose(wc, (2, 1, 4, 6, 5, 3, 0))
        ).reshape(PART, WFREE)
        in_maps.append({"xs": xs_k, "w": w_k})
    return in_maps


def _unpack_outputs(results):
    out = np.empty((B, O, H, W), dtype=np.float32)
    for k in range(NCORES):
        arr = results[k]["out"].reshape(4, 16, HL, 16, O)  # [j, b, w, pbl, o]
        strip = np.transpose(arr, (1, 4, 2, 3, 0)).reshape(B, O, HL, W)
        out[:, :, 8 * k:8 * (k + 1), :] = strip
    return out


def run(x, weights, trace=False):
    if "nc" not in _CACHE:
        _CACHE["nc"] = _build()
    nc = _CACHE["nc"]
    in_maps = _pack_inputs(np.asarray(x), np.asarray(weights))
    res = run_bass_kernel_spmd(nc, in_maps, list(range(NCORES)), trace=trace)
    return _unpack_outputs(res.results), res


def kernel(x, weights):
    out, _ = run(x, weights)
    return out


# revision 8
# speedup vs baseline: 39.7869x; 3.2564x over previous
"""Locally-connected 3x3 block (LCBlock) Trainium2 kernel.

Computes out = ELU(einsum('ocdkij,bcdkij->boij', weights, unfold(x)))
for x:[16,32,64,64] f32, weights:[32,32,3,3,64,64] f32.

Strategy (8 NeuronCores, SPMD, no collectives):
  - Spatially shard H=64 into 8 strips of 8 rows; each core gets its strip's
    per-position weights (they shard perfectly) and a 10-row halo'd slab of x.
  - Per position p=(y,x) the LC contraction is a tiny matmul
    [B=16, CK=288] x [CK=288, O=32].  We run it on the PE as 3 PSUM-accumulated
    matmuls (one per dj kernel column): lhsT = patch [K=96=(3di x 32c), M=16b]
    (cheap LDWEIGHTS: cost scales with columns=16), rhs = weights
    [96, 32o] (the big tensor streams as the moving operand).  4 positions run
    concurrently in the 4 PE column-groups via tile_position.
  - bf16 operands (fp32 PSUM accumulation) halve the HBM roofline.
  - ELU = max(x, exp(min(x,0))-1): 2 DVE ops + 1 ACT op per row-wave.
Host side packs/scatters inputs and gathers the 8 output strips.
"""

import os
import sys

import numpy as np

for _p in ("/opt/trn_rl_repo", "/root/.axon_site/_ro/trn_rl_repo"):
    if os.path.isdir(_p) and _p not in sys.path:
        sys.path.insert(0, _p)

import ml_dtypes

import concourse.bacc as bacc
import concourse.mybir as mybir
import concourse.tile as tile
from concourse.bass_interp import get_hw_module
from concourse.bass_utils import run_bass_kernel_spmd

BF16 = ml_dtypes.bfloat16

# Problem shape (hardcoded per contract).
B, C, O, H, W = 16, 32, 32, 64, 64
NCORES = 8
HL = H // NCORES  # local rows per core
KW = 3  # conv kernel size
PART = KW * C  # 96 partitions: (di, c)
XW = W + 2  # padded row width
XFREE = HL * XW * B  # x slab free elems/partition
WCH = 4 * 16 * KW * O  # weight elems/partition per row-wave (j, pbl, dj, o)
WFREE = HL * WCH
OUTF = HL * 16 * O  # out free elems/partition: (w, pbl, o)

_CACHE = {}


def _build(hw=True, reps=1, variant="full"):
    nc = bacc.Bacc(
        "TRN2", target_bir_lowering=False, debug=False, num_devices=NCORES
    )
    xs_d = nc.dram_tensor("xs", [PART, XFREE], mybir.dt.bfloat16, kind="ExternalInput")
    w_d = nc.dram_tensor("w", [PART, WFREE], mybir.dt.bfloat16, kind="ExternalInput")
    out_d = nc.dram_tensor("out", [4, 16, OUTF], mybir.dt.float32, kind="ExternalOutput")

    with tile.TileContext(nc) as tc:
        with (
            tc.tile_pool(name="xp", bufs=1) as xp,
            tc.tile_pool(name="wp", bufs=3) as wp,
            tc.tile_pool(name="pp", bufs=3, space="PSUM") as pp,
            tc.tile_pool(name="op", bufs=1) as op,
            tc.tile_pool(name="tp", bufs=2) as tp,
        ):
          for _rep in range(reps):
            x_t = xp.tile([PART, XFREE], mybir.dt.bfloat16, tag="x")
            nc.sync.dma_start(x_t[:], xs_d[:])
            out_t = op.tile([128, OUTF], mybir.dt.float32, tag="o")

            for wv in range(HL):  # one image row per wave
                w_t = wp.tile([PART, WCH], mybir.dt.bfloat16, tag="w")
                nc.sync.dma_start(w_t[:], w_d[:][:, wv * WCH:(wv + 1) * WCH])
                ps = pp.tile([128, 512], mybir.dt.float32, tag="ps")
                # init rows the col-tiled matmuls never touch (ELU reads all 128)
                nc.vector.memset(ps[:], 0.0)
                if variant != "dma_only":
                    for pbl in range(16):
                        for j in range(4):
                            roff = (j * 16 + pbl) * 3 * O
                            for dj in range(KW):
                                lo = (wv * XW + pbl * 4 + j + dj) * B
                                nc.tensor.matmul(
                                    ps[32 * j:32 * j + B, pbl * 32:(pbl + 1) * 32],
                                    x_t[:, lo:lo + B],
                                    w_t[:, roff + dj * O:roff + (dj + 1) * O],
                                    start=(dj == 0),
                                    stop=(dj == KW - 1),
                                    tile_position=(0, 32 * j),
                                )
                if variant in ("full",):
                    # ELU: out = max(psum, exp(min(psum, 0)) - 1)
                    t1 = tp.tile([128, 512], mybir.dt.float32, tag="t1")
                    nc.vector.tensor_scalar_min(t1[:], ps[:], 0.0)
                    nc.scalar.activation(
                        t1[:], t1[:], mybir.ActivationFunctionType.Exp
                    )
                    nc.vector.scalar_tensor_tensor(
                        out_t[:, wv * 512:(wv + 1) * 512],
                        t1[:],
                        -1.0,
                        ps[:],
                        op0=mybir.AluOpType.add,
                        op1=mybir.AluOpType.max,
                    )
                else:
                    # cheap evacuation so deps/out exist: copy psum -> out
                    nc.vector.tensor_copy(
                        out_t[:, wv * 512:(wv + 1) * 512], ps[:]
                    )
            oap = out_d.ap()
            for j in range(4):
                nc.sync.dma_start(oap[j], out_t[32 * j:32 * j + 16, :])

    nc.compile()
    if hw:
        nc.m = get_hw_module(nc.m)
    return nc


def _pack_inputs(x, weights):
    """Host-side scatter: per-core bf16 slabs."""
    xpad = np.pad(x, ((0, 0), (0, 0), (1, 1), (1, 1))).astype(BF16)  # [B,C,66,66]
    wb = np.asarray(weights).astype(BF16)  # [O,C,3,3,H,W]
    in_maps = []
    for k in range(NCORES):
        # x slab: [di*32+c, y, xx, b] = xpad[b, c, 8k+y+di, xx]
        slabs = [
            np.transpose(xpad[:, :, 8 * k + di:8 * k + di + HL, :], (1, 2, 3, 0))
            for di in range(KW)
        ]
        xs_k = np.ascontiguousarray(np.stack(slabs, 0)).reshape(PART, XFREE)
        # weights: [di*32+c, w, j, pbl, dj, o] = W[o, c, di, dj, 8k+w, pbl*4+j]
        wc = wb[:, :, :, :, 8 * k:8 * (k + 1), :].reshape(O, C, KW, KW, HL, 16, 4)
        w_k = np.ascontiguousarray(
            np.transpose(wc, (2, 1, 4, 6, 5, 3, 0))
        ).reshape(PART, WFREE)
        in_maps.append({"xs": xs_k, "w": w_k})
    return in_maps


def _unpack_outputs(results):
    out = np.empty((B, O, H, W), dtype=np.float32)
    for k in range(NCORES):
        arr = results[k]["out"].reshape(4, 16, HL, 16, O)  # [j, b, w, pbl, o]
        strip = np.transpose(arr, (1, 4, 2, 3, 0)).reshape(B, O, HL, W)
        out[:, :, 8 * k:8 * (k + 1), :] = strip
    return out


def run(x, weights, trace=False):
    if "nc" not in _CACHE:
        _CACHE["nc"] = _build()
    nc = _CACHE["nc"]
    in_maps = _pack_inputs(np.asarray(x), np.asarray(weights))
    res = run_bass_kernel_spmd(nc, in_maps, list(range(NCORES)), trace=trace)
    return _unpack_outputs(res.results), res


def kernel(x, weights):
    out, _ = run(x, weights)
    return out
